# revision 7
# baseline (speedup 1.0000x reference)
"""Multi-head attention (B=2, S=2048, D=1024, H=16) on 8 trn2 NeuronCores.

Sharding: data-parallel over batch (2) x tensor-parallel over head-groups (4).
Core c handles batch b = c // 4 and heads [4g, 4g+4) with g = c % 4.
Each core computes q/k in transposed layout [ch, S], v in [S, ch] layout,
per-head scores^T = k_h @ q_h^T on the PE (K=64), exp on the scalar engine
(1/8 scale folded in; no max-subtraction needed for N(0,1) scores), the
attention output via an ones-augmented V (M=65: 64 dims + denominator row),
normalization with a partition-broadcast DMA of the reciprocal denominators,
and finally a row-parallel partial of the output projection. The host sums
the 4 partials per batch element and adds the bias.

All matmuls run as float32r (tf32-like, 1 cycle/row on the PE; ~1.5e-4 rel
err per matmul measured on HW) accumulating in fp32 PSUM.
"""

import numpy as np

import concourse.bass as bass
import concourse.tile as tile
from concourse import bacc, mybir
from concourse.bass_utils import run_bass_kernel_spmd

P = 128
S = 2048          # sequence length
D = 1024          # model dim
HD = 64           # head dim
HL = 4            # heads per core
CL = HL * HD      # 256 local channels
DC = D // P       # 8 contraction chunks
ST = S // P       # 16 seq tiles
NCORES = 8
GROUPS = 4

f32 = mybir.dt.float32
f32r = mybir.dt.float32r
FT = mybir.ActivationFunctionType

_CACHE = {}


def _attn_head(nc, psp, ohp, expp, rbp, q_t, k_t, v_aug, attnT, ct, hh, rb_dram):
    h = ct * 2 + hh
    co = hh * HD
    qh = q_t[co : co + HD, :]
    kh = k_t[co : co + HD, :]

    oh = ohp.tile([HD + 1, S], f32, tag="oh")
    for kt in range(ST):
        ex = expp.tile([P, S], f32r, tag="exp")
        for half in range(2):
            ps = psp.tile([P, 1024], f32, tag="ps")
            for j in range(2):
                nc.tensor.matmul(
                    ps[:, j * 512 : (j + 1) * 512],
                    kh[:, kt * P : (kt + 1) * P],
                    qh[:, half * 1024 + j * 512 : half * 1024 + (j + 1) * 512],
                    start=True,
                    stop=True,
                    skip_group_check=True,
                )
            # exp(scores / 8); evacuates PSUM and rounds to f32r in one op
            nc.scalar.activation(
                ex[:, half * 1024 : (half + 1) * 1024],
                ps[:],
                FT.Exp,
                scale=0.125,
            )
        for nch in range(4):
            nc.tensor.matmul(
                oh[:, nch * 512 : (nch + 1) * 512],
                v_aug[:, kt, h, :],
                ex[:, nch * 512 : (nch + 1) * 512],
                start=(kt == 0),
                stop=(kt == ST - 1),
                skip_group_check=True,
            )

    # oh rows 0..63 = unnormalized head output^T, row 64 = softmax denominators
    rb = rbp.tile([HD + 1, S], f32, tag="rb")
    nc.vector.reciprocal(rb[HD : HD + 1, :], oh[HD : HD + 1, :])
    # broadcast the reciprocal row across 64 partitions: SBUF partition APs
    # can't have stride 0, so bounce through DRAM and broadcast on the read
    w_i = nc.sync.dma_start(rb_dram[0:1, :], rb[HD : HD + 1, :])
    dram_ap = rb_dram[0:1, :]
    bcast_src = bass.AP(
        tensor=dram_ap.tensor,
        offset=dram_ap.offset,
        ap=[[0, HD]] + list(dram_ap.ap[1:]),
    )
    r_i = nc.gpsimd.dma_start(rb[0:HD, :], bcast_src)
    tile.add_dep_helper(r_i.ins, w_i.ins, sync=True, reason="rb dram bounce")
    nc.vector.tensor_mul(attnT[co : co + HD, ct, :], oh[0:HD, :], rb[0:HD, :])


def _build():
    nc = bacc.Bacc(None, target_bir_lowering=False)
    xT = nc.dram_tensor("xT", [D, S], f32r, kind="ExternalInput")
    wqT = nc.dram_tensor("wqT", [D, CL], f32r, kind="ExternalInput")
    wkT = nc.dram_tensor("wkT", [D, CL], f32r, kind="ExternalInput")
    wvT = nc.dram_tensor("wvT", [D, CL], f32r, kind="ExternalInput")
    woT = nc.dram_tensor("woT", [CL, D], f32r, kind="ExternalInput")
    outp = nc.dram_tensor("outp", [S, D], f32, kind="ExternalOutput")
    rb_drams = [
        nc.dram_tensor(f"rbd{h}", [1, S], f32, kind="Internal") for h in range(HL)
    ]

    with tile.TileContext(nc) as tc:
        with (
            tc.tile_pool(name="main", bufs=1) as main,
            tc.tile_pool(name="qk", bufs=2) as qkp,
            tc.tile_pool(name="exp", bufs=2) as expp,
            tc.tile_pool(name="rb", bufs=1) as rbp,
            tc.tile_pool(name="ob", bufs=2) as obp,
            tc.tile_pool(name="ps", bufs=2, space="PSUM") as psp,
            tc.tile_pool(name="oh", bufs=1, space="PSUM") as ohp,
        ):
            v_aug = main.tile([P, ST, HL, HD + 1], f32r)
            ones_sb = main.tile([P, ST, HL, 1], f32)
            nc.vector.memset(ones_sb[:], 1.0)
            nc.vector.tensor_copy(v_aug[:, :, :, HD : HD + 1], ones_sb[:])
            attnT = main.tile([P, 2, S], f32r)
            wo_sb = main.tile([P, 2, D], f32r)
            for cc in range(2):
                nc.sync.dma_start(wo_sb[:, cc, :], woT[cc * P : (cc + 1) * P, :])

            with tc.tile_pool(name="w", bufs=1) as wp:
                x_sb = wp.tile([P, DC, S], f32r)
                wq_sb = wp.tile([P, DC, CL], f32r)
                wk_sb = wp.tile([P, DC, CL], f32r)
                wv_sb = wp.tile([P, DC, CL], f32r)
                for dc in range(DC):
                    sl = slice(dc * P, (dc + 1) * P)
                    nc.sync.dma_start(x_sb[:, dc, :], xT[sl, :])
                    nc.sync.dma_start(wq_sb[:, dc, :], wqT[sl, :])
                    nc.sync.dma_start(wk_sb[:, dc, :], wkT[sl, :])
                    nc.sync.dma_start(wv_sb[:, dc, :], wvT[sl, :])

                # V projection: v[s, c] accumulated over d-chunks
                for st in range(ST):
                    pv = psp.tile([P, CL], f32, tag="ps")
                    for dc in range(DC):
                        nc.tensor.matmul(
                            pv[:],
                            x_sb[:, dc, st * P : (st + 1) * P],
                            wv_sb[:, dc, :],
                            start=(dc == 0),
                            stop=(dc == DC - 1),
                        )
                    nc.vector.tensor_copy(
                        v_aug[:, st, :, 0:HD],
                        pv[:].rearrange("p (h d) -> p h d", h=HL),
                    )

                # Q^T / K^T projections for both channel-tiles
                q_tiles = []
                k_tiles = []
                for ct in range(2):
                    q_t = qkp.tile([P, S], f32r, tag="q")
                    k_t = qkp.tile([P, S], f32r, tag="k")
                    q_tiles.append(q_t)
                    k_tiles.append(k_t)
                    for w_sb, dst in ((wq_sb, q_t), (wk_sb, k_t)):
                        for nch in range(4):
                            pq = psp.tile([P, 512], f32, tag="ps")
                            for dc in range(DC):
                                nc.tensor.matmul(
                                    pq[:],
                                    w_sb[:, dc, ct * P : (ct + 1) * P],
                                    x_sb[:, dc, nch * 512 : (nch + 1) * 512],
                                    start=(dc == 0),
                                    stop=(dc == DC - 1),
                                )
                            nc.vector.tensor_copy(
                                dst[:, nch * 512 : (nch + 1) * 512], pq[:]
                            )

            for ct in range(2):
                for hh in range(2):
                    _attn_head(
                        nc, psp, ohp, expp, rbp,
                        q_tiles[ct], k_tiles[ct], v_aug, attnT, ct, hh,
                        rb_drams[ct * 2 + hh],
                    )

            # Output projection partial: out[s, :] += attn^T.T @ woT
            for st in range(ST):
                po = psp.tile([P, D], f32, tag="ps")
                for cc in range(2):
                    for j in range(2):
                        nc.tensor.matmul(
                            po[:, j * 512 : (j + 1) * 512],
                            attnT[:, cc, st * P : (st + 1) * P],
                            wo_sb[:, cc, j * 512 : (j + 1) * 512],
                            start=(cc == 0),
                            stop=(cc == 1),
                            skip_group_check=True,
                        )
                ob = obp.tile([P, D], f32, tag="ob")
                nc.scalar.copy(ob[:], po[:])
                nc.sync.dma_start(outp[st * P : (st + 1) * P, :], ob[:])

    nc.compile()
    return nc


def _get_nc():
    if "nc" not in _CACHE:
        _CACHE["nc"] = _build()
    return _CACHE["nc"]


def _make_in_maps(x, Wq, Wk, Wv, Wo, bo=None):
    x = np.asarray(x)
    Wq, Wk, Wv, Wo = (np.asarray(a) for a in (Wq, Wk, Wv, Wo))
    in_maps = []
    xTs = [np.ascontiguousarray(x[b].T) for b in range(x.shape[0])]
    for c in range(NCORES):
        b, g = divmod(c, GROUPS)
        sl = slice(g * CL, (g + 1) * CL)
        in_maps.append(
            {
                "xT": xTs[b],
                "wqT": np.ascontiguousarray(Wq[sl].T),
                "wkT": np.ascontiguousarray(Wk[sl].T),
                "wvT": np.ascontiguousarray(Wv[sl].T),
                "woT": np.ascontiguousarray(Wo[:, sl].T),
            }
        )
    return in_maps


def kernel(x, Wq, Wk, Wv, Wo, bo):
    x = np.asarray(x)
    bo = np.asarray(bo)
    B = x.shape[0]
    assert x.shape == (2, S, D)

    nc = _get_nc()
    in_maps = _make_in_maps(x, Wq, Wk, Wv, Wo)
    res = run_bass_kernel_spmd(nc, in_maps, core_ids=list(range(NCORES)))
    out = np.empty((B, S, D), np.float32)
    for b in range(B):
        acc = res.results[4 * b]["outp"].astype(np.float32)
        for g in range(1, GROUPS):
            acc = acc + res.results[4 * b + g]["outp"]
        out[b] = acc + bo[None, :]
    return out


# revision 32
# speedup vs baseline: 2199.4290x; 2199.4290x over previous
"""Multi-head attention (B=2, S=2048, D=1024, H=16) on 8 trn2 NeuronCores.

Sharding: data-parallel over batch (2) x tensor-parallel over head-groups (4).
Core c handles batch b = c // 4 and heads [4g, 4g+4) with g = c % 4.

Per core: q/k computed in transposed layout [ch, S], v in [S, ch] layout with
an appended ones column per head; per-head scores^T = k_h @ q_h^T on the PE
(K=64), exp on the scalar engine (1/8 scale folded in; scores are ~N(0,1) so
no max-subtraction is needed), attention output via the ones-augmented V
(M=65: 64 dims + softmax-denominator row) accumulated in PSUM per q-half,
normalization via reciprocal + DRAM-bounce partition-broadcast + DVE mul, and
a row-parallel output projection emitted as two per-channel-chunk partials.
The host sums the 8 partials per batch element and adds the bias.

All matmuls run as float32r (tf32-like, 1 PE cycle/row at moving dim >= 256;
~1.5e-4 rel err per matmul measured on HW) accumulating in fp32 PSUM.

Scheduling: the attention phase is ACT(exp)-bound, so PE-only work is
interleaved into it — the ct=1 q/k projections ride inside heads 0/1 and the
first output-projection half inside heads 2/3 — using a 2-bank aux PSUM pool
left free by the q-half split (scores 2x[128,1024]=4 banks, oh [65,1024]=2).
"""

import numpy as np

import concourse.bass as bass
import concourse.tile as tile
from concourse import bacc, mybir
from concourse.bass_utils import run_bass_kernel_spmd

P = 128
S = 2048          # sequence length
D = 1024          # model dim
HD = 64           # head dim
HL = 4            # heads per core
CL = HL * HD      # 256 local channels
DC = D // P       # 8 contraction chunks
ST = S // P       # 16 seq tiles
QH = 1024         # q-half width
NCORES = 8
GROUPS = 4

f32 = mybir.dt.float32
f32r = mybir.dt.float32r
FT = mybir.ActivationFunctionType

_CACHE = {}


def _attn_head(nc, pools, q_t, k_t, v_aug, attnT_cc, ct, hh, fillers, rb_drams,
               stride=4, fillers_q1=None, stride_q1=2):
    """Emit one head's attention. fillers: closures popped every `stride` kt
    iterations to fill PE slack during the ACT-bound exp pipeline;
    fillers_q1 are only consumed during the second q-pass (for work that
    depends on this head's first-pass output)."""
    psp, ohp, expp, rbp = pools["ps"], pools["oh"], pools["exp"], pools["rb"]
    h = ct * 2 + hh
    co = hh * HD
    qh = q_t[co : co + HD, :]
    kh = k_t[co : co + HD, :]

    DEPTH = 6  # scores/exp run DEPTH kt ahead of attnV so the normalize
    # chain of the previous q-pass (which blocks attnV kt0 via the single
    # oh buffer) never starves the ACT exp pipeline: PE is in-order, so
    # the prefix must be emitted before the first attnV.
    it = 0
    for qpass in range(2):
        oh = ohp.tile([HD + 1, QH], f32, tag="oh")
        exs = {}

        def emit_scores_exp(kt):
            ps = psp.tile([P, QH], f32, tag="ps", name="ps")
            for j in range(2):
                nc.tensor.matmul(
                    ps[:, j * 512 : (j + 1) * 512],
                    kh[:, kt * P : (kt + 1) * P],
                    qh[:, qpass * QH + j * 512 : qpass * QH + (j + 1) * 512],
                    start=True,
                    stop=True,
                    skip_group_check=True,
                )
            ex = expp.tile([P, QH], f32r, tag="exp", name="ex")
            nc.scalar.activation(ex[:], ps[:], FT.Exp, scale=0.125)
            exs[kt] = ex

        for kt in range(DEPTH):
            emit_scores_exp(kt)
        for kt in range(ST):
            if kt + DEPTH < ST:
                emit_scores_exp(kt + DEPTH)
            ex = exs.pop(kt)
            for j in range(2):
                nc.tensor.matmul(
                    oh[:, j * 512 : (j + 1) * 512],
                    v_aug[:, kt, h, :],
                    ex[:, j * 512 : (j + 1) * 512],
                    start=(kt == 0),
                    stop=(kt == ST - 1),
                    skip_group_check=True,
                )
            it += 1
            if fillers and it % stride == 0:
                fillers.pop(0)()
            if qpass == 1 and fillers_q1 and kt % stride_q1 == 0:
                fillers_q1.pop(0)()

        # normalize: rows 0..63 /= row 64 (softmax denominators).
        # reciprocal of the denominator row, partition-broadcast via a DRAM
        # bounce (SBUF partition APs can't have stride 0), then DVE mul.
        # Kept off the in-order PE stream so only the oh release couples.
        rbt = rbp.tile([HD + 1, QH], f32, tag="rb", name="rbt")
        nc.vector.reciprocal(rbt[HD : HD + 1, :], oh[HD : HD + 1, :])
        rbd = rb_drams[h * 2 + qpass]
        w_i = nc.sync.dma_start(rbd[0:1, :], rbt[HD : HD + 1, :])
        dram_ap = rbd[0:1, :]
        bcast_src = bass.AP(
            tensor=dram_ap.tensor,
            offset=dram_ap.offset,
            ap=[[0, HD]] + list(dram_ap.ap[1:]),
        )
        r_i = nc.gpsimd.dma_start(rbt[0:HD, :], bcast_src)
        tile.add_dep_helper(r_i.ins, w_i.ins, sync=True, reason="rb dram bounce")
        nc.vector.tensor_mul(
            attnT_cc[co : co + HD, qpass * QH : (qpass + 1) * QH],
            oh[0:HD, :],
            rbt[0:HD, :],
        )


def _build(reps=1):
    nc = bacc.Bacc(None, target_bir_lowering=False)
    xT = nc.dram_tensor("xT", [D, S], f32r, kind="ExternalInput")
    wqT = nc.dram_tensor("wqT", [D, CL], f32r, kind="ExternalInput")
    wkT = nc.dram_tensor("wkT", [D, CL], f32r, kind="ExternalInput")
    wvT = nc.dram_tensor("wvT", [D, CL], f32r, kind="ExternalInput")
    woT = nc.dram_tensor("woT", [CL, D], f32r, kind="ExternalInput")
    outs = [
        nc.dram_tensor(f"outp{j}", [S, D], f32, kind="ExternalOutput")
        for j in range(2)
    ]

    with tile.TileContext(nc) as tc:
        for rep in range(reps):
            _emit_body(nc, tc, xT, wqT, wkT, wvT, woT, outs, rep)
    nc.compile()
    return nc


def _emit_body(nc, tc, xT, wqT, wkT, wvT, woT, outs, rep):
    rb_drams = [
        nc.dram_tensor(f"rbd_{rep}_{i}", [1, QH], f32, kind="Internal")
        for i in range(8)
    ]
    if True:
        with (
            tc.tile_pool(name="main", bufs=1) as main,
            tc.tile_pool(name="qk", bufs=2) as qkp,
            tc.tile_pool(name="exp", bufs=8) as expp,
            tc.tile_pool(name="rb", bufs=2) as rbp,
            tc.tile_pool(name="ob", bufs=3) as obp,
            tc.tile_pool(name="ps", bufs=2, space="PSUM") as psp,
            tc.tile_pool(name="oh", bufs=1, space="PSUM") as ohp,
            tc.tile_pool(name="aux", bufs=2, space="PSUM") as auxp,
        ):
            pools = {"ps": psp, "oh": ohp, "exp": expp, "rb": rbp, "aux": auxp}

            v_aug = main.tile([P, ST, HL, HD + 1], f32r)
            ones_sb = main.tile([P, ST, HL, 1], f32)
            nc.vector.memset(ones_sb[:], 1.0)
            nc.vector.tensor_copy(v_aug[:, :, :, HD : HD + 1], ones_sb[:])
            ones64f = main.tile([P, HD], f32)
            nc.vector.memset(ones64f[:], 1.0)
            ones64 = main.tile([P, HD], f32r)
            nc.vector.tensor_copy(ones64[:], ones64f[:])
            pools["ones64"] = ones64
            attnT0 = main.tile([P, S], f32r, tag="attnT0")
            attnT1 = main.tile([P, S], f32r, tag="attnT1")
            attnT = [attnT0, attnT1]
            wo_sb = main.tile([P, 2, D], f32r)

            def emit_wo(cc, st):
                def go():
                    for j in range(2):
                        po = auxp.tile([P, 512], f32, tag="aux")
                        nc.tensor.matmul(
                            po[:],
                            attnT[cc][:, st * P : (st + 1) * P],
                            wo_sb[:, cc, j * 512 : (j + 1) * 512],
                            start=True,
                            stop=True,
                            skip_group_check=True,
                        )
                        ob = obp.tile([P, 512], f32, tag="ob")
                        nc.vector.tensor_copy(ob[:], po[:])
                        nc.sync.dma_start(
                            outs[cc][st * P : (st + 1) * P, j * 512 : (j + 1) * 512],
                            ob[:],
                        )
                return go

            with tc.tile_pool(name="w", bufs=1) as wp:
                x_sb = wp.tile([P, DC, S], f32r)
                wq_sb = wp.tile([P, DC, CL], f32r)
                wk_sb = wp.tile([P, DC, CL], f32r)
                wv_sb = wp.tile([P, DC, CL], f32r)
                # DMA order: wv first (v-projection starts immediately),
                # x col-block 0, then wq/wk, then the rest of x, then wo.
                for dc in range(DC):
                    nc.sync.dma_start(wv_sb[:, dc, :], wvT[dc * P : (dc + 1) * P, :])
                xTr = xT[:].rearrange("(c p) s -> p c s", p=P)
                nc.sync.dma_start(x_sb[:, :, 0:512], xTr[:, :, 0:512])
                for dc in range(DC):
                    sl = slice(dc * P, (dc + 1) * P)
                    nc.sync.dma_start(wq_sb[:, dc, :], wqT[sl, :])
                    nc.sync.dma_start(wk_sb[:, dc, :], wkT[sl, :])
                for blk in range(1, 4):
                    nc.sync.dma_start(
                        x_sb[:, :, blk * 512 : (blk + 1) * 512],
                        xTr[:, :, blk * 512 : (blk + 1) * 512],
                    )
                for cc in range(2):
                    nc.sync.dma_start(wo_sb[:, cc, :], woT[cc * P : (cc + 1) * P, :])

                # V projection: v[s, c] accumulated over d-chunks
                for st in range(ST):
                    pv = psp.tile([P, CL], f32, tag="ps")
                    for dc in range(DC):
                        nc.tensor.matmul(
                            pv[:],
                            x_sb[:, dc, st * P : (st + 1) * P],
                            wv_sb[:, dc, :],
                            start=(dc == 0),
                            stop=(dc == DC - 1),
                            skip_group_check=True,
                        )
                    nc.vector.tensor_copy(
                        v_aug[:, st, :, 0:HD],
                        pv[:].rearrange("p (h d) -> p h d", h=HL),
                    )

                def emit_proj(w_sb, dst, ct, nch):
                    def go():
                        pq = auxp.tile([P, 512], f32, tag="aux")
                        for dc in range(DC):
                            nc.tensor.matmul(
                                pq[:],
                                w_sb[:, dc, ct * P : (ct + 1) * P],
                                x_sb[:, dc, nch * 512 : (nch + 1) * 512],
                                start=(dc == 0),
                                stop=(dc == DC - 1),
                                skip_group_check=True,
                            )
                        nc.vector.tensor_copy(
                            dst[:, nch * 512 : (nch + 1) * 512], pq[:]
                        )
                    return go

                # ct=0 q/k projections up front (on the main ps pool)
                q_tiles, k_tiles = [], []
                for ct in range(2):
                    q_tiles.append(qkp.tile([P, S], f32r, tag="q", name=f"q{ct}"))
                    k_tiles.append(qkp.tile([P, S], f32r, tag="k", name=f"k{ct}"))
                for w_sb, dst in ((wq_sb, q_tiles[0]), (wk_sb, k_tiles[0])):
                    for nch in range(4):
                        pq = psp.tile([P, 512], f32, tag="ps")
                        for dc in range(DC):
                            nc.tensor.matmul(
                                pq[:],
                                w_sb[:, dc, 0:P],
                                x_sb[:, dc, nch * 512 : (nch + 1) * 512],
                                start=(dc == 0),
                                stop=(dc == DC - 1),
                                skip_group_check=True,
                            )
                        nc.vector.tensor_copy(dst[:, nch * 512 : (nch + 1) * 512], pq[:])

                # ct=1 q/k projections ride inside head 0 as fillers
                fillers = [
                    emit_proj(w_sb, dst, 1, nch)
                    for (w_sb, dst) in ((wq_sb, q_tiles[1]), (wk_sb, k_tiles[1]))
                    for nch in range(4)
                ]
                _attn_head(nc, pools, q_tiles[0], k_tiles[0], v_aug, attnT[0],
                           0, 0, fillers, rb_drams)
                # Wo0 for the first q-half (st 0..7 = seq cols 0..1023) only
                # needs h0+h1 first-pass outputs -> ride h1's second q-pass
                _attn_head(nc, pools, q_tiles[0], k_tiles[0], v_aug, attnT[0],
                           0, 1, fillers, rb_drams,
                           fillers_q1=[emit_wo(0, st) for st in range(8)])
                assert not fillers

            # heads 2/3: rest of Wo0 rides h2, Wo1 first half rides h3 pass 2
            fillers = [emit_wo(0, st) for st in range(8, ST)]
            _attn_head(nc, pools, q_tiles[1], k_tiles[1], v_aug, attnT[1],
                       1, 0, fillers, rb_drams)
            _attn_head(nc, pools, q_tiles[1], k_tiles[1], v_aug, attnT[1],
                       1, 1, fillers, rb_drams,
                       fillers_q1=[emit_wo(1, st) for st in range(8)])
            assert not fillers

            # second half of Wo1 (tail)
            for st in range(8, ST):
                emit_wo(1, st)()


def _get_nc():
    if "nc" not in _CACHE:
        _CACHE["nc"] = _build()
    return _CACHE["nc"]


def _make_in_maps(x, Wq, Wk, Wv, Wo, bo=None):
    x = np.asarray(x)
    Wq, Wk, Wv, Wo = (np.asarray(a) for a in (Wq, Wk, Wv, Wo))
    in_maps = []
    xTs = [np.ascontiguousarray(x[b].T) for b in range(x.shape[0])]
    for c in range(NCORES):
        b, g = divmod(c, GROUPS)
        sl = slice(g * CL, (g + 1) * CL)
        in_maps.append(
            {
                "xT": xTs[b],
                "wqT": np.ascontiguousarray(Wq[sl].T),
                "wkT": np.ascontiguousarray(Wk[sl].T),
                "wvT": np.ascontiguousarray(Wv[sl].T),
                "woT": np.ascontiguousarray(Wo[:, sl].T),
            }
        )
    return in_maps


def kernel(x, Wq, Wk, Wv, Wo, bo):
    x = np.asarray(x)
    bo = np.asarray(bo)
    B = x.shape[0]
    assert x.shape == (2, S, D)

    nc = _get_nc()
    in_maps = _make_in_maps(x, Wq, Wk, Wv, Wo)
    res = run_bass_kernel_spmd(nc, in_maps, core_ids=list(range(NCORES)))
    out = np.empty((B, S, D), np.float32)
    for b in range(B):
        acc = res.results[4 * b]["outp0"].astype(np.float32)
        acc = acc + res.results[4 * b]["outp1"]
        for g in range(1, GROUPS):
            acc = acc + res.results[4 * b + g]["outp0"]
            acc = acc + res.results[4 * b + g]["outp1"]
        out[b] = acc + bo[None, :]
    return out
